# revision 1
# baseline (speedup 1.0000x reference)
"""Trainium2 Bass kernel for nn_CombineLoss (focal + dice + edge loss).

Sharding: data-parallel over the batch dim B=8 -> one batch image per
NeuronCore.  Each core computes 13 partial sums per 128-row tile (8 tiles);
the host combines them in float64.

Math notes (per head; heads = 2 softmaxed prediction heads + 1 raw-prob head):
  - softmax over C=2 channels:  p1 = sigmoid(x1-x0), p0 = sigmoid(x0-x1),
    so log p_t = log(sigmoid((2t-1)*(x1-x0))).
  - 7x7 "circular" box filter (29 taps, weight 1/29) is computed on the
    TensorEngine as 7 horizontally shifted banded matmuls over the partition
    (row) axis; the -29*t term is folded into the dx=0 band so PSUM ends up
    holding 29*(ave - t) and |PSUM| = 29*at.
  - per-tile sums ride on ScalarE activation accum_out and VectorE
    affine_mul_reduce accum_out; final reduction happens on host.
  - elementwise path runs in bf16 (at/t planes are small integers -> exact;
    lp/pt lose ~0.4% per element which cancels in the 8M-element means).
"""

import numpy as np

# ---------------------------------------------------------------------------
# problem constants (hardcoded per contest rules)
N_HEADS_PRED = 2   # predictions list dim
B = 8              # batch -> one per core
C = 2
H = 1024
W = 1024
P = 128            # partitions
NT = H // P        # 8 row tiles per image
TBW = 1032         # padded width of the bf16 target slots (3 left, 5 right)
SLOTS = 13         # stat slots per tile
NSLOT = NT * SLOTS
EPS = 1e-10
N_CORES = 8

# stat slot layout (per tile, base = 13*i):
#  +0  sum(t)
#  +1  pred0 sum(pt)       +2  pred0 sum(lp)
#  +3  pred1 sum(pt)       +4  pred1 sum(lp)
#  +5  diss  sum(lp)       +6  diss  sum(p1)
#  +7  pred0 sum(lp*at29)  +8  pred0 sum(pt*t)
#  +9  pred1 sum(lp*at29)  +10 pred1 sum(pt*t)
#  +11 diss  sum(lp*at29)  +12 diss  sum(p1*t)

_CACHE = {}


def _ensure_path():
    try:
        import concourse  # noqa: F401
    except ImportError:
        import sys
        for p in ("/opt/trn_rl_repo", "/root/.axon_site/_ro/trn_rl_repo"):
            if p not in sys.path:
                sys.path.insert(0, p)


def _make_bands():
    """7 [128,128] banded matrices, bf16 (all entries small ints -> exact)."""
    import ml_dtypes
    idx = np.arange(P)
    dy = idx[:, None] - idx[None, :]  # dy[p, m] = p - m
    b7c = (np.abs(dy) <= 3).astype(np.float32) - 29.0 * (dy == 0)
    b7p = (dy >= 125).astype(np.float32)
    b7n = (dy <= -125).astype(np.float32)
    b5c = (np.abs(dy) <= 2).astype(np.float32)
    b5p = (dy >= 126).astype(np.float32)
    b5n = (dy <= -126).astype(np.float32)
    i128 = (dy == 0).astype(np.float32)
    bands = np.stack([b7c, b7p, b7n, b5c, b5p, b5n, i128])
    return bands.astype(ml_dtypes.bfloat16)


B7C, B7P, B7N, B5C, B5P, B5N, I128 = range(7)


def _build_nc():
    _ensure_path()
    import concourse.mybir as mybir
    from concourse import bacc
    from concourse.tile import TileContext

    f32 = mybir.dt.float32
    bf16 = mybir.dt.bfloat16
    i32 = mybir.dt.int32
    Alu = mybir.AluOpType
    Act = mybir.ActivationFunctionType

    nc = bacc.Bacc()
    preds = nc.dram_tensor("preds", [N_HEADS_PRED, C, H, W], f32,
                           kind="ExternalInput")
    diss = nc.dram_tensor("diss", [C, H, W], f32, kind="ExternalInput")
    target = nc.dram_tensor("target", [H, W], i32, kind="ExternalInput")
    bands = nc.dram_tensor("bands", [7, P, P], bf16, kind="ExternalInput")
    stats_out = nc.dram_tensor("stats", [P, NSLOT], f32, kind="ExternalOutput")

    with TileContext(nc) as tc:
        with (
            tc.tile_pool(name="const", bufs=1) as constp,
            tc.tile_pool(name="res", bufs=1) as resp,
            tc.tile_pool(name="io", bufs=6) as iop,
            tc.tile_pool(name="mid", bufs=2) as midp,
            tc.tile_pool(name="ps", bufs=2, space="PSUM") as psump,
        ):
            bands_sb = constp.tile([P, 7, P], bf16)
            for k in range(7):
                nc.sync.dma_start(out=bands_sb[:, k, :], in_=bands[k])
            stats_sb = constp.tile([P, NSLOT], f32)
            neg_one = constp.tile([P, 1], f32)
            nc.gpsimd.memset(neg_one[:], -1.0)
            eps_t = constp.tile([P, 1], f32)
            nc.gpsimd.memset(eps_t[:], EPS)
            tb_all = resp.tile([P, NT, TBW], bf16)
            sgn_all = resp.tile([P, NT, W], bf16)
            t_all = resp.tile([P, NT, W], i32)
            nc.gpsimd.memset(tb_all[:], 0.0)

            # ---- phase A: target -> bf16 {0,1} planes + sign planes -------
            for i in range(NT):
                t_i = t_all[:, i, :]
                nc.sync.dma_start(out=t_i, in_=target[i * P:(i + 1) * P, :])
                nc.scalar.activation(
                    tb_all[:, i, 3:3 + W], t_i, Act.Copy,
                    accum_out=stats_sb[:, i * SLOTS:i * SLOTS + 1])
                nc.vector.tensor_scalar(
                    sgn_all[:, i, :], t_i, 2.0, -1.0,
                    Alu.mult, Alu.add)

            # ---- phase B ---------------------------------------------------
            for i in range(NT):
                base = i * SLOTS
                sgn_i = sgn_all[:, i, :]
                tmask = t_all[:, i, :]

                # conv on PE: psum = 29*ave - 29*t  (integers, exact)
                psum = psump.tile([P, W], f32)
                for h in range(2):
                    w0 = h * 512
                    mms = [(B7C, 0, i)]
                    if i > 0:
                        mms.append((B7P, 0, i - 1))
                    if i < NT - 1:
                        mms.append((B7N, 0, i + 1))
                    for dx in (-1, 1, -2, 2):
                        mms.append((B5C, dx, i))
                        if i > 0:
                            mms.append((B5P, dx, i - 1))
                        if i < NT - 1:
                            mms.append((B5N, dx, i + 1))
                    mms.append((I128, -3, i))
                    mms.append((I128, 3, i))
                    for j, (bk, dx, ti) in enumerate(mms):
                        nc.tensor.matmul(
                            psum[:, w0:w0 + 512],
                            bands_sb[:, bk, :],
                            tb_all[:, ti, 3 + dx + w0: 3 + dx + w0 + 512],
                            start=(j == 0), stop=(j == len(mms) - 1))

                at = midp.tile([P, W], bf16, tag="at")
                nc.scalar.activation(at, psum, Act.Abs)  # at = 29*|t-ave|

                tb_i = tb_all[:, i, 3:3 + W]
                xt0 = iop.tile([P, C, W], f32, tag="xin")
                xt1 = iop.tile([P, C, W], f32, tag="xin")
                dt = iop.tile([P, C, W], f32, tag="xin")
                nc.sync.dma_start(
                    out=xt0, in_=preds[0, :, i * P:(i + 1) * P, :].rearrange(
                        "c h w -> h c w"))
                nc.sync.dma_start(
                    out=xt1, in_=preds[1, :, i * P:(i + 1) * P, :].rearrange(
                        "c h w -> h c w"))
                nc.sync.dma_start(
                    out=dt, in_=diss[:, i * P:(i + 1) * P, :].rearrange(
                        "c h w -> h c w"))

                d0 = midp.tile([P, W], bf16, tag="d")
                nc.vector.tensor_tensor(d0, xt0[:, 1, :], xt0[:, 0, :],
                                        Alu.subtract)
                s0 = midp.tile([P, W], bf16, tag="s")
                nc.vector.tensor_tensor(s0, d0, sgn_i, Alu.mult)
                d1 = midp.tile([P, W], bf16, tag="d")
                nc.vector.tensor_tensor(d1, xt1[:, 1, :], xt1[:, 0, :],
                                        Alu.subtract)
                s1 = midp.tile([P, W], bf16, tag="s")
                nc.vector.tensor_tensor(s1, d1, sgn_i, Alu.mult)
                ptd = midp.tile([P, W], f32, tag="ptd")
                nc.vector.tensor_copy(ptd, dt[:, 0, :])
                nc.vector.copy_predicated(ptd, tmask, dt[:, 1, :])

                # ACT ops grouped by function to avoid act-table reloads
                pt0 = midp.tile([P, W], bf16, tag="pt")
                nc.scalar.activation(pt0, s0, Act.Sigmoid,
                                     accum_out=stats_sb[:, base + 1:base + 2])
                pt1 = midp.tile([P, W], bf16, tag="pt")
                nc.scalar.activation(pt1, s1, Act.Sigmoid,
                                     accum_out=stats_sb[:, base + 3:base + 4])
                lp0 = midp.tile([P, W], bf16, tag="lp")
                nc.scalar.activation(lp0, pt0, Act.Ln, bias=eps_t[:, 0:1],
                                     accum_out=stats_sb[:, base + 2:base + 3])
                lp1 = midp.tile([P, W], bf16, tag="lp")
                nc.scalar.activation(lp1, pt1, Act.Ln, bias=eps_t[:, 0:1],
                                     accum_out=stats_sb[:, base + 4:base + 5])
                lpd = midp.tile([P, W], bf16, tag="lp")
                nc.scalar.activation(lpd, ptd, Act.Ln, bias=eps_t[:, 0:1],
                                     accum_out=stats_sb[:, base + 5:base + 6])
                scr3 = midp.tile([P, W], f32, tag="scr")
                nc.scalar.activation(scr3, dt[:, 1, :], Act.Copy,
                                     accum_out=stats_sb[:, base + 6:base + 7])

                # fused product sums on DVE (slots 8/10/12 hold sum(x*t))
                scr = midp.tile([P, W], bf16, tag="scrb")
                nc.vector.affine_mul_reduce(
                    out=scr, accum_out=stats_sb[:, base + 7:base + 8],
                    in0=lp0, in1=at, scale=1.0, bias=0.0)
                scr2 = midp.tile([P, W], bf16, tag="scrb")
                nc.vector.affine_mul_reduce(
                    out=scr2, accum_out=stats_sb[:, base + 8:base + 9],
                    in0=pt0, in1=tb_i, scale=1.0, bias=0.0)
                scr4 = midp.tile([P, W], bf16, tag="scrb")
                nc.vector.affine_mul_reduce(
                    out=scr4, accum_out=stats_sb[:, base + 9:base + 10],
                    in0=lp1, in1=at, scale=1.0, bias=0.0)
                scr5 = midp.tile([P, W], bf16, tag="scrb")
                nc.vector.affine_mul_reduce(
                    out=scr5, accum_out=stats_sb[:, base + 10:base + 11],
                    in0=pt1, in1=tb_i, scale=1.0, bias=0.0)
                scr6 = midp.tile([P, W], bf16, tag="scrb")
                nc.vector.affine_mul_reduce(
                    out=scr6, accum_out=stats_sb[:, base + 11:base + 12],
                    in0=lpd, in1=at, scale=1.0, bias=0.0)
                scr7 = midp.tile([P, W], f32, tag="scr")
                nc.vector.affine_mul_reduce(
                    out=scr7, accum_out=stats_sb[:, base + 12:base + 13],
                    in0=dt[:, 1, :], in1=tb_i, scale=1.0, bias=0.0)

            nc.sync.dma_start(out=stats_out[:], in_=stats_sb[:])

    nc.finalize()
    return nc


def get_program():
    if "nc" not in _CACHE:
        _CACHE["nc"] = _build_nc()
    return _CACHE["nc"]


def make_in_maps(predictions, Diss, target):
    bands = _make_bands()
    in_maps = []
    for c in range(N_CORES):
        in_maps.append({
            "preds": np.ascontiguousarray(predictions[:, c], dtype=np.float32),
            "diss": np.ascontiguousarray(Diss[0, c], dtype=np.float32),
            "target": np.ascontiguousarray(target[c], dtype=np.int32),
            "bands": bands,
        })
    return in_maps


def assemble(stats_list, sigma, diff):
    """Combine per-core [128, 104] stats into the scalar loss (float64)."""
    HW = float(H * W)
    n_heads = 3
    focal_sum = np.zeros(n_heads, np.float64)
    edge_sum = np.zeros(n_heads, np.float64)
    dice_ratio = np.zeros(n_heads, np.float64)
    for st in stats_list:
        g = st.astype(np.float64).sum(axis=0).reshape(NT, SLOTS).sum(axis=0)
        s_t = g[0]
        n0 = HW - s_t
        # heads 0,1: pred; head 2: diss
        for hh in range(3):
            if hh < 2:
                spt, slp = g[1 + 2 * hh], g[2 + 2 * hh]
                e29, i_sum = g[7 + 2 * hh], g[8 + 2 * hh]
                sp1 = n0 + 2.0 * i_sum - spt
            else:
                slp, sp1 = g[5], g[6]
                e29, i_sum = g[11], g[12]
            u_sum = sp1 + s_t
            focal_sum[hh] += -slp
            edge_sum[hh] += -e29 / 29.0
            dice_ratio[hh] += 2.0 * i_sum / (u_sum + EPS)

    sig2 = np.asarray(sigma, np.float64) ** 2
    denom = float(N_CORES) * HW
    loss = 0.0
    for hh in range(n_heads):
        focal = focal_sum[hh] / denom
        dice = 1.0 - dice_ratio[hh] / float(N_CORES)
        edge = edge_sum[hh] / denom
        loss += focal / sig2[0] + dice / sig2[1] + edge / sig2[2]
    loss += float(diff)
    loss += float(np.sum(np.log(sig2))) / 2.0
    return np.float32(loss)


def run_on_hw(predictions, Diss, target, trace=False):
    _ensure_path()
    from concourse.bass_utils import run_bass_kernel_spmd
    nc = get_program()
    in_maps = make_in_maps(predictions, Diss, target)
    res = run_bass_kernel_spmd(nc, in_maps, list(range(N_CORES)), trace=trace)
    stats_list = [r["stats"] for r in res.results]
    return stats_list, res


def kernel(predictions, Diss, target, diff, sigma):
    predictions = np.asarray(predictions)
    Diss = np.asarray(Diss)
    target = np.asarray(target)
    stats_list, _ = run_on_hw(predictions, Diss, target, trace=False)
    return assemble(stats_list, np.asarray(sigma), np.asarray(diff))



# revision 3
# speedup vs baseline: 1.1656x; 1.1656x over previous
"""Trainium2 Bass kernel for nn_CombineLoss (focal + dice + edge), v3.

Baseline architecture (proven ops only) with three reductions in work:
  - Inputs host-cast to bf16 (RNE, unbiased): halves HBM traffic and makes
    every DVE tensor_tensor op 2x-eligible (all operands 2-byte).
  - Combined log: lp0+lp1+lpd = ln(pt0*pt1*ptd + eps) -> ONE Ln per tile
    instead of three, and one AMR for the edge sum instead of three (the
    three heads share sigma weights, so only combined sums are needed).
  - Target planes (padded bf16 conv plane, sign plane, int16 mask) prepared
    on host; phase A reduces to DMAs.  Sum(t) computed directly on host.

Conv: unchanged baseline scheme - 7x7 circular mask as banded bf16 matmuls
with the -29*t term folded into the dx=0 band; PSUM holds 29*(ave-t).

Stat slots per tile (base = 8*i):
  +0 Sum pt0   +1 Sum pt1   +2 Sum L   +3 Sum(L*at29)
  +4 Sum(pt0*t)  +5 Sum(pt1*t)  +6 Sum(d1*t)  +7 Sum d1
"""

import numpy as np

N_HEADS_PRED = 2
B = 8
C = 2
H = 1024
W = 1024
P = 128
NT = H // P
TBW = 1032           # padded bf16 target plane (3 left, 5 right)
SLOTS = 8
NSLOT = NT * SLOTS
EPS = 1e-10
N_CORES = 8

_CACHE = {}


def _ensure_path():
    try:
        import concourse  # noqa: F401
    except ImportError:
        import sys
        for p in ("/opt/trn_rl_repo", "/root/.axon_site/_ro/trn_rl_repo"):
            if p not in sys.path:
                sys.path.insert(0, p)


def _make_bands():
    """7 [128,128] banded matrices, bf16 (small ints -> exact)."""
    import ml_dtypes
    idx = np.arange(P)
    dy = idx[:, None] - idx[None, :]
    b7c = (np.abs(dy) <= 3).astype(np.float32) - 29.0 * (dy == 0)
    b7p = (dy >= 125).astype(np.float32)
    b7n = (dy <= -125).astype(np.float32)
    b5c = (np.abs(dy) <= 2).astype(np.float32)
    b5p = (dy >= 126).astype(np.float32)
    b5n = (dy <= -126).astype(np.float32)
    i128 = (dy == 0).astype(np.float32)
    bands = np.stack([b7c, b7p, b7n, b5c, b5p, b5n, i128])
    return bands.astype(ml_dtypes.bfloat16)


B7C, B7P, B7N, B5C, B5P, B5N, I128 = range(7)


def _build_nc():
    _ensure_path()
    import concourse.mybir as mybir
    from concourse import bacc
    from concourse.tile import TileContext

    f32 = mybir.dt.float32
    bf16 = mybir.dt.bfloat16
    i16 = mybir.dt.int16
    Alu = mybir.AluOpType
    Act = mybir.ActivationFunctionType

    nc = bacc.Bacc()
    preds = nc.dram_tensor("preds", [N_HEADS_PRED, C, H, W], bf16,
                           kind="ExternalInput")
    diss = nc.dram_tensor("diss", [C, H, W], bf16, kind="ExternalInput")
    tbpad_d = nc.dram_tensor("tbpad", [H, TBW], bf16, kind="ExternalInput")
    sgn_d = nc.dram_tensor("sgn", [H, W], bf16, kind="ExternalInput")
    tb16_d = nc.dram_tensor("tb16", [H, W], bf16, kind="ExternalInput")
    tmask_d = nc.dram_tensor("tmask", [H, W], i16, kind="ExternalInput")
    bands = nc.dram_tensor("bands", [7, P, P], bf16, kind="ExternalInput")
    stats_out = nc.dram_tensor("stats", [P, NSLOT], f32, kind="ExternalOutput")

    with TileContext(nc) as tc:
        with (
            tc.tile_pool(name="const", bufs=1) as constp,
            tc.tile_pool(name="res", bufs=1) as resp,
            tc.tile_pool(name="io", bufs=4) as iop,
            tc.tile_pool(name="mid", bufs=2) as midp,
            tc.tile_pool(name="ps", bufs=2, space="PSUM") as psump,
        ):
            bands_sb = constp.tile([P, 7, P], bf16)
            for k in range(7):
                nc.sync.dma_start(out=bands_sb[:, k, :], in_=bands[k])
            stats_sb = constp.tile([P, NSLOT], f32)
            eps_t = constp.tile([P, 1], f32)
            nc.gpsimd.memset(eps_t[:], EPS)

            tb_all = resp.tile([P, NT, TBW], bf16)
            tb16_all = resp.tile([P, NT, W], bf16)
            sgn_all = resp.tile([P, NT, W], bf16)
            tmask_all = resp.tile([P, NT, W], i16)

            def load_tplanes(i):
                rows = slice(i * P, (i + 1) * P)
                nc.sync.dma_start(out=tb_all[:, i, :], in_=tbpad_d[rows, :])
                nc.sync.dma_start(out=tb16_all[:, i, :], in_=tb16_d[rows, :])
                nc.sync.dma_start(out=sgn_all[:, i, :], in_=sgn_d[rows, :])
                nc.sync.dma_start(out=tmask_all[:, i, :], in_=tmask_d[rows, :])

            for i in range(3):
                load_tplanes(i)

            def conv(i):
                # conv on PE: psum = 29*ave - 29*t (integers, exact)
                psum = psump.tile([P, W], f32)
                for h in range(2):
                    w0 = h * 512
                    mms = [(B7C, 0, i)]
                    if i > 0:
                        mms.append((B7P, 0, i - 1))
                    if i < NT - 1:
                        mms.append((B7N, 0, i + 1))
                    for dx in (-1, 1, -2, 2):
                        mms.append((B5C, dx, i))
                        if i > 0:
                            mms.append((B5P, dx, i - 1))
                        if i < NT - 1:
                            mms.append((B5N, dx, i + 1))
                    mms.append((I128, -3, i))
                    mms.append((I128, 3, i))
                    for j, (bk, dx, ti) in enumerate(mms):
                        nc.tensor.matmul(
                            psum[:, w0:w0 + 512],
                            bands_sb[:, bk, :],
                            tb_all[:, ti, 3 + dx + w0: 3 + dx + w0 + 512],
                            start=(j == 0), stop=(j == len(mms) - 1))
                return psum

            def front(i, psum):
                base = i * SLOTS
                sgn_i = sgn_all[:, i, :]
                x = iop.tile([P, 4, W], bf16, tag="x")
                nc.sync.dma_start(out=x[:, 0, :],
                                  in_=preds[0, 0, i * P:(i + 1) * P, :])
                nc.sync.dma_start(out=x[:, 1, :],
                                  in_=preds[0, 1, i * P:(i + 1) * P, :])
                nc.sync.dma_start(out=x[:, 2, :],
                                  in_=preds[1, 0, i * P:(i + 1) * P, :])
                nc.sync.dma_start(out=x[:, 3, :],
                                  in_=preds[1, 1, i * P:(i + 1) * P, :])
                dt = iop.tile([P, 2, W], bf16, tag="dt")
                nc.sync.dma_start(out=dt[:, 0, :],
                                  in_=diss[0, i * P:(i + 1) * P, :])
                nc.sync.dma_start(out=dt[:, 1, :],
                                  in_=diss[1, i * P:(i + 1) * P, :])

                # ACT: at = 29*|t-ave| (Abs/Copy live in every table)
                at = midp.tile([P, W], bf16, tag="at")
                nc.scalar.activation(at, psum, Act.Abs)

                d0 = midp.tile([P, W], bf16, tag="d")
                nc.vector.tensor_tensor(d0, x[:, 1, :], x[:, 0, :],
                                        Alu.subtract)
                s0 = midp.tile([P, W], bf16, tag="s")
                nc.vector.tensor_tensor(s0, d0, sgn_i, Alu.mult)
                pt0 = midp.tile([P, W], bf16, tag="pt0")
                nc.scalar.activation(pt0, s0, Act.Sigmoid,
                                     accum_out=stats_sb[:, base + 0:base + 1])
                d1 = midp.tile([P, W], bf16, tag="d")
                nc.vector.tensor_tensor(d1, x[:, 3, :], x[:, 2, :],
                                        Alu.subtract)
                s1 = midp.tile([P, W], bf16, tag="s")
                nc.vector.tensor_tensor(s1, d1, sgn_i, Alu.mult)
                pt1 = midp.tile([P, W], bf16, tag="pt1")
                nc.scalar.activation(pt1, s1, Act.Sigmoid,
                                     accum_out=stats_sb[:, base + 1:base + 2])
                # Sum d1 rides an ACT Copy (Copy in sigmoid table: no reload)
                scr0 = midp.tile([P, W], bf16, tag="scr0")
                nc.scalar.activation(scr0, dt[:, 1, :], Act.Copy,
                                     accum_out=stats_sb[:, base + 7:base + 8])

                ptd = midp.tile([P, W], bf16, tag="ptd")
                nc.vector.tensor_copy(ptd, dt[:, 0, :])
                nc.vector.copy_predicated(ptd, tmask_all[:, i, :], dt[:, 1, :])
                mm = midp.tile([P, W], bf16, tag="m")
                nc.vector.tensor_tensor(mm, pt0, pt1, Alu.mult)
                m2 = midp.tile([P, W], bf16, tag="m2")
                nc.vector.tensor_tensor(m2, mm, ptd, Alu.mult)
                return dt, at, pt0, pt1, m2

            def back(i, dt, at, pt0, pt1, m2):
                base = i * SLOTS
                tb_i = tb16_all[:, i, :]
                ll = midp.tile([P, W], bf16, tag="ll")
                nc.scalar.activation(ll, m2, Act.Ln, bias=eps_t[:, 0:1],
                                     accum_out=stats_sb[:, base + 2:base + 3])

                scr1 = midp.tile([P, W], bf16, tag="scr1")
                nc.vector.affine_mul_reduce(
                    out=scr1, accum_out=stats_sb[:, base + 3:base + 4],
                    in0=ll, in1=at, scale=1.0, bias=0.0)
                scr2 = midp.tile([P, W], bf16, tag="scr2")
                nc.vector.affine_mul_reduce(
                    out=scr2, accum_out=stats_sb[:, base + 4:base + 5],
                    in0=pt0, in1=tb_i, scale=1.0, bias=0.0)
                scr3 = midp.tile([P, W], bf16, tag="scr3")
                nc.vector.affine_mul_reduce(
                    out=scr3, accum_out=stats_sb[:, base + 5:base + 6],
                    in0=pt1, in1=tb_i, scale=1.0, bias=0.0)
                scr4 = midp.tile([P, W], bf16, tag="scr4")
                nc.vector.affine_mul_reduce(
                    out=scr4, accum_out=stats_sb[:, base + 6:base + 7],
                    in0=dt[:, 1, :], in1=tb_i, scale=1.0, bias=0.0)

            for ip in range(0, NT, 2):
                i, j2 = ip, ip + 1
                psi = conv(i)
                psj = conv(j2)
                fi = front(i, psi)
                if i + 3 < NT:
                    load_tplanes(i + 3)
                fj = front(j2, psj)
                if i + 4 < NT:
                    load_tplanes(i + 4)
                back(i, *fi)
                back(j2, *fj)

            nc.sync.dma_start(out=stats_out[:], in_=stats_sb[:])

    nc.finalize()
    return nc


def get_program():
    if "nc" not in _CACHE:
        _CACHE["nc"] = _build_nc()
    return _CACHE["nc"]


def make_in_maps(predictions, Diss, target):
    import ml_dtypes
    bf = ml_dtypes.bfloat16
    bands = _make_bands()
    in_maps = []
    for c in range(N_CORES):
        t = target[c]
        tbpad = np.zeros((H, TBW), bf)
        tbpad[:, 3:3 + W] = t
        in_maps.append({
            "preds": predictions[:, c].astype(bf),
            "diss": Diss[0, c].astype(bf),
            "tbpad": tbpad,
            "sgn": (2 * t - 1).astype(bf),
            "tb16": t.astype(bf),
            "tmask": t.astype(np.int16),
            "bands": bands,
        })
    return in_maps


def assemble(stats_list, target, sigma, diff):
    """Combine per-core [128, NSLOT] stats into the scalar loss (float64)."""
    HW = float(H * W)
    focal_sum = 0.0
    edge_sum = 0.0
    dice_ratio = np.zeros(3, np.float64)
    for c, st in enumerate(stats_list):
        g = st.astype(np.float64).sum(axis=0).reshape(NT, SLOTS).sum(axis=0)
        s_t = float(target[c].sum())
        n0 = HW - s_t
        focal_sum += -g[2]
        edge_sum += -g[3] / 29.0
        for hh in range(2):
            spt = g[0 + hh]
            i_sum = g[4 + hh]
            sp1 = n0 + 2.0 * i_sum - spt
            dice_ratio[hh] += 2.0 * i_sum / (sp1 + s_t + EPS)
        dice_ratio[2] += 2.0 * g[6] / (g[7] + s_t + EPS)

    sig2 = np.asarray(sigma, np.float64) ** 2
    denom = float(N_CORES) * HW
    focal = focal_sum / denom
    edge = edge_sum / denom
    dice = 3.0 - dice_ratio.sum() / float(N_CORES)
    loss = (focal / sig2[0] + dice / sig2[1] + edge / sig2[2]
            + float(diff) + float(np.sum(np.log(sig2))) / 2.0)
    return np.float32(loss)


def run_on_hw(predictions, Diss, target, trace=False):
    _ensure_path()
    from concourse.bass_utils import run_bass_kernel_spmd
    nc = get_program()
    in_maps = make_in_maps(predictions, Diss, target)
    res = run_bass_kernel_spmd(nc, in_maps, list(range(N_CORES)), trace=trace)
    stats_list = [r["stats"] for r in res.results]
    return stats_list, res


def kernel(predictions, Diss, target, diff, sigma):
    predictions = np.asarray(predictions)
    Diss = np.asarray(Diss)
    target = np.asarray(target)
    stats_list, _ = run_on_hw(predictions, Diss, target, trace=False)
    return assemble(stats_list, target, np.asarray(sigma), np.asarray(diff))
